# revision 1
# baseline (speedup 1.0000x reference)
"""Causal self-attention (B=4, T=2048, C=768, H=12) on 8 TRN2 NeuronCores.

Sharding: (batch x head-half). Core c handles batch b = c//2 and heads
hg*6..hg*6+5 where hg = c%2. Each core computes qkv projection for its
1152 W_attn columns, causal attention for its 6 heads, and a partial
c_proj using its 384 rows of W_proj. Host sums the pair partials + bias.

Device-side layout trick: qkv is computed directly in transposed form
(qkvT = W.T @ x.T), so Q^T/K^T land in [64, T] per head (the layout the
S^T matmul wants) and V is computed in natural [T, 64] layout with an
extra ones-column, so the P^T @ V' matmul yields both O^T and the
softmax row-sums in one pass. The causal mask is applied by zeroing
exp(S^T) above the diagonal with gpsimd.affine_select. Per-query
normalization: reciprocal_approx_fast on the sums row, partition
broadcast, and a fused multiply during the PSUM->SBUF evacuation.
"""

import sys

import numpy as np

try:
    import concourse  # noqa: F401
except ImportError:
    sys.path.insert(0, "/opt/trn_rl_repo")

B, T, C, H, D = 4, 2048, 768, 12, 64
NH = H // 2          # heads per core
CH = NH * D          # 384 channels per core
NCB = C // 128       # 6 contraction blocks
NTB = T // 128       # 16 t-blocks
NQC = T // 512       # 4 query chunks
VW = D + 1           # 65: V plus ones column

_CACHE = {}


def _build_nc(probes=False, reps=1, phases=3):
    from concourse import bacc, mybir, tile

    f32 = mybir.dt.float32
    f32r = mybir.dt.float32r
    bf16 = mybir.dt.bfloat16
    AF = mybir.ActivationFunctionType
    ALU = mybir.AluOpType

    nc = bacc.Bacc("TRN2", target_bir_lowering=False, debug=False, num_devices=8)
    if probes:
        dbg_qkT_d = nc.dram_tensor("dbg_qkT", [128, 6, T], bf16, kind="ExternalOutput")
        dbg_v1_d = nc.dram_tensor("dbg_v1", [128, NTB, NH * VW], f32, kind="ExternalOutput")
        dbg_pt_d = nc.dram_tensor("dbg_pt", [2, 128, 1024], f32, kind="ExternalOutput")
        dbg_rbb_d = nc.dram_tensor("dbg_rbb", [NH, 64, 512], f32, kind="ExternalOutput")
        dbg_opv_d = nc.dram_tensor("dbg_opv", [NH, VW, 512], f32, kind="ExternalOutput")
        dbg_ot_d = nc.dram_tensor("dbg_ot", [NH, 64, 512], f32, kind="ExternalOutput")

    xt_d = nc.dram_tensor("xt", [C, T], bf16, kind="ExternalInput")
    wqk_d = nc.dram_tensor("wqk", [C, 2 * CH], bf16, kind="ExternalInput")
    wv_d = nc.dram_tensor("wv", [C, CH], bf16, kind="ExternalInput")
    bqk_d = nc.dram_tensor("bqk", [6, 128], f32, kind="ExternalInput")
    bv_d = nc.dram_tensor("bv", [1, CH], f32, kind="ExternalInput")
    wp_d = nc.dram_tensor("wp", [64, NH * C], bf16, kind="ExternalInput")
    out_d = nc.dram_tensor("out", [T, C], f32, kind="ExternalOutput")

    with tile.TileContext(nc) as tc:
        with (
            tc.tile_pool(name="const", bufs=1) as cp,
            tc.tile_pool(name="work", bufs=2) as wk,
            tc.tile_pool(name="pt", bufs=4) as ptp,
            tc.tile_pool(name="ot", bufs=8) as otp,
            tc.tile_pool(name="outs", bufs=2) as osp,
            tc.tile_pool(name="ps_s", bufs=3, space="PSUM") as ps_s,
            tc.tile_pool(name="ps_o", bufs=2, space="PSUM") as ps_o,
        ):
          if True:
            # ---- resident inputs (loaded once, outside the rep loop) ----
            # (per-c-block DMAs in separate tiles so the first qkv matmuls
            # only wait on block 0, not the whole transfer)
            xt_r = xt_d.rearrange("(n p) m -> n p m", p=128)
            wqk_r = wqk_d.rearrange("(n p) m -> n p m", p=128)
            wv_r = wv_d.rearrange("(n p) m -> n p m", p=128)
            xt_t, wqk_t, wv_t = [], [], []
            for ci in range(NCB):
                t_ = cp.tile([128, T], bf16, tag=f"xt{ci}")
                nc.sync.dma_start(out=t_, in_=xt_r[ci])
                xt_t.append(t_)
                t_ = cp.tile([128, 2 * CH], bf16, tag=f"wqk{ci}")
                nc.sync.dma_start(out=t_, in_=wqk_r[ci])
                wqk_t.append(t_)
                t_ = cp.tile([128, CH], bf16, tag=f"wv{ci}")
                nc.sync.dma_start(out=t_, in_=wv_r[ci])
                wv_t.append(t_)
            wp_sb = cp.tile([64, NH, C], bf16, tag="wp")
            nc.sync.dma_start(out=wp_sb, in_=wp_d.rearrange("p (n m) -> p n m", n=NH))
            bqk_sb = cp.tile([128, 6], f32, tag="bqk")
            nc.sync.dma_start(out=bqk_sb, in_=bqk_d.rearrange("n p -> p n"))
            bv_sb = cp.tile([1, CH], f32, tag="bv")
            nc.sync.dma_start(out=bv_sb, in_=bv_d.ap())
            bvb_sb = cp.tile([128, CH], f32, tag="bvb")
            nc.gpsimd.partition_broadcast(bvb_sb, bv_sb)
            # ones row at partition 64 for the K=1 sums-broadcast matmul
            ones64 = cp.tile([VW, 64], f32r, tag="ones64")
            nc.vector.memset(ones64.bitcast(f32), 1.0)
            # 0/1 causal band mask: band[k, c] = 1.0 iff c - k >= 512.
            # Slice [512-delta : 512-delta+w] masks a diagonal block with
            # offset delta (valid iff q - k >= delta).
            band = cp.tile([128, 640], bf16, tag="band")
            nc.vector.memset(band, 1.0)
            nc.gpsimd.affine_select(
                out=band,
                in_=band,
                compare_op=ALU.is_ge,
                fill=0.0,
                base=-512,
                pattern=[[1, 640]],
                channel_multiplier=-1,
            )

          for _rep in range(reps):
            # ---- outputs of the qkv projection ----
            qkT = cp.tile([128, 6, T], bf16, tag="qkT")  # rows: Q blocks 0-2, K blocks 3-5
            v1 = cp.tile([128, NTB, NH * VW], bf16, tag="v1")
            v1_4d = v1.rearrange("p n (h e) -> p n h e", e=VW)
            nc.vector.memset(v1_4d[:, :, :, D], 1.0)  # ones columns

            # V natural: [t 128-block, 384] = sum_c xT[c, t].T @ Wv[c, :]
            for tb in range(NTB):
                psv = ps_o.tile([128, 512], f32, tag="o")
                for ci in range(NCB):
                    nc.tensor.matmul(
                        psv[:, 0:CH],
                        lhsT=xt_t[ci][:, tb * 128:(tb + 1) * 128],
                        rhs=wv_t[ci],
                        start=(ci == 0),
                        stop=(ci == NCB - 1),
                    )
                psv_3d = psv[:, 0:CH].rearrange("p (h e) -> p h e", e=D)
                bvb_3d = bvb_sb.rearrange("p (h e) -> p h e", e=D)
                nc.vector.tensor_add(v1_4d[:, tb, :, 0:D], psv_3d, bvb_3d)

            # qkvT: Q^T and K^T, [c_out 128-block, t] = sum_c W[c, c_out] * xT[c, t]
            for co in (0, 3, 1, 4, 2, 5):
                for j in range(NQC):
                    ps = ps_s.tile([128, 1024], f32, tag="s")
                    for ci in range(NCB):
                        nc.tensor.matmul(
                            ps[:, 0:512],
                            lhsT=wqk_t[ci][:, co * 128:(co + 1) * 128],
                            rhs=xt_t[ci][:, j * 512:(j + 1) * 512],
                            start=(ci == 0),
                            stop=(ci == NCB - 1),
                        )
                    # evac with bias (+0.125 scale folded into Q); writes bf16
                    # (on DVE to keep ScalarE free for the exp stream)
                    nc.vector.tensor_scalar(
                        out=qkT[:, co, j * 512:(j + 1) * 512],
                        in0=ps[:, 0:512],
                        scalar1=0.125 if co < 3 else 1.0,
                        scalar2=bqk_sb[:, co:co + 1],
                        op0=ALU.mult,
                        op1=ALU.add,
                    )

            if phases == 1:  # timing-only: qkv phase alone
                nc.sync.dma_start(out=out_d[0:128, :], in_=qkT[:, 0, 0:1536].bitcast(f32))
                nc.sync.dma_start(
                    out=out_d[128:256, 0:NH * VW // 2 - 1], in_=v1[:, 0, 0:NH * VW - 2].bitcast(f32)
                )
                continue
            # ---- attention + projection, per query chunk ----
            for j in range(NQC):
                ot_tiles = []
                for h in range(NH):
                    ht, hp = h // 2, (h % 2) * 64
                    nkb = 4 * (j + 1)  # causal 128-key-blocks
                    ps_pv = ps_o.tile([128, 512], f32, tag="o")
                    for pti in range(nkb // 2):
                        ps = ps_s.tile([128, 1024], f32, tag="s")
                        for half in range(2):
                            kb = pti * 2 + half
                            nc.tensor.matmul(
                                ps[:, half * 512:(half + 1) * 512],
                                lhsT=qkT[hp:hp + 64, 3 + ht, kb * 128:(kb + 1) * 128],
                                rhs=qkT[hp:hp + 64, ht, j * 512:(j + 1) * 512],
                                start=True,
                                stop=True,
                            )
                        pt = ptp.tile([128, 1024], bf16, tag="pt")
                        nc.scalar.activation(out=pt, in_=ps, func=AF.Exp)
                        _dump_pt = probes and j == 0 and h == 0
                        for half in range(2):
                            kb = pti * 2 + half
                            delta = kb * 128 - j * 512
                            if delta >= 0:  # diagonal block: zero q < k + delta
                                w = delta + 128
                                nc.vector.tensor_mul(
                                    pt[:, half * 512:half * 512 + w],
                                    pt[:, half * 512:half * 512 + w],
                                    band[:, 512 - delta:512 - delta + w],
                                )
                        if _dump_pt:
                            nc.sync.dma_start(
                                out=dbg_pt_d[pti], in_=pt
                            )
                        for half in range(2):
                            kb = pti * 2 + half
                            nc.tensor.matmul(
                                ps_pv[0:VW, :],
                                lhsT=v1_4d[:, kb, h, :],
                                rhs=pt[:, half * 512:(half + 1) * 512],
                                start=(kb == 0),
                                stop=(kb == nkb - 1),
                            )
                    if probes and j == 0:
                        opv_sb = wk.tile([VW, 512], f32, tag="opv")
                        nc.scalar.copy(opv_sb, ps_pv[0:VW, :])
                        nc.sync.dma_start(out=dbg_opv_d[h], in_=opv_sb)
                    # normalize: O^T[d, q] / sums[q]. gpsimd/custom-DVE ops
                    # misread APs at base partition 64, so: evacuate psum to
                    # SBUF (frees the PV slot), broadcast the sums row to 64
                    # partitions with a K=1 PE matmul (base-64 lhsT/rhs is
                    # fine for the PE), then reciprocal at base 0.
                    ov = wk.tile([VW, 512], f32r, tag="ov")
                    nc.vector.tensor_copy(ov, ps_pv[0:VW, :])
                    ps_b = ps_o.tile([128, 512], f32, tag="o")
                    nc.tensor.matmul(
                        ps_b[0:64, :],
                        lhsT=ones64[D:VW, :],
                        rhs=ov[D:VW, :],
                        start=True,
                        stop=True,
                    )
                    rbb = wk.tile([64, 512], f32, tag="rbb")
                    nc.vector.reciprocal_approx_fast(out=rbb, in_=ps_b[0:64, :])
                    ot = otp.tile([64, 512], bf16, tag="ot")
                    nc.vector.tensor_mul(ot, ov[0:D, :], rbb)
                    ot_tiles.append(ot)
                    if probes and j == 0:
                        nc.sync.dma_start(out=dbg_rbb_d[h], in_=rbb)
                        pass

                if phases == 2:  # timing-only: no projection
                    for h in range(NH):
                        nc.sync.dma_start(
                            out=out_d[j * 512 + h * 64:j * 512 + (h + 1) * 64, 0:256],
                            in_=ot_tiles[h].bitcast(f32),
                        )
                    continue
                # partial c_proj for this chunk's 4 t-blocks
                for tb4 in range(4):
                    tb = j * 4 + tb4
                    ost = osp.tile([128, C], f32, tag="ost")
                    for lo, n in ((0, 512), (512, 256)):
                        psp = ps_o.tile([128, 512], f32, tag="o")
                        for h in range(NH):
                            nc.tensor.matmul(
                                psp[:, 0:n],
                                lhsT=ot_tiles[h][:, tb4 * 128:(tb4 + 1) * 128],
                                rhs=wp_sb[:, h, lo:lo + n],
                                start=(h == 0),
                                stop=(h == NH - 1),
                            )
                        nc.vector.tensor_copy(ost[:, lo:lo + n], psp[:, 0:n])
                    nc.sync.dma_start(
                        out=out_d[tb * 128:(tb + 1) * 128, :], in_=ost
                    )

            if probes:
                nc.sync.dma_start(out=dbg_qkT_d.ap(), in_=qkT)
                pass

    nc.compile()
    return nc


def _bf16(a):
    import ml_dtypes
    return np.ascontiguousarray(a).astype(ml_dtypes.bfloat16)


def _shard_inputs(x, W_attn, b_attn, W_proj):
    in_maps = []
    for c in range(8):
        b, hg = c // 2, c % 2
        q0, k0, v0 = hg * CH, C + hg * CH, 2 * C + hg * CH
        bqk = np.concatenate(
            [b_attn[q0:q0 + CH] * 0.125, b_attn[k0:k0 + CH]]
        ).reshape(6, 128)
        in_maps.append({
            "xt": _bf16(x[b].T),
            "wqk": _bf16(np.concatenate(
                [W_attn[:, q0:q0 + CH], W_attn[:, k0:k0 + CH]], axis=1)),
            "wv": _bf16(W_attn[:, v0:v0 + CH]),
            "bqk": np.ascontiguousarray(bqk, dtype=np.float32),
            "bv": np.ascontiguousarray(
                b_attn[v0:v0 + CH].reshape(1, CH), dtype=np.float32
            ),
            "wp": _bf16(
                W_proj[hg * CH:(hg + 1) * CH, :]
                .reshape(NH, 64, C)
                .transpose(1, 0, 2)
                .reshape(64, NH * C)
            ),
        })
    return in_maps


def kernel(x, W_attn, b_attn, W_proj, b_proj, _trace=False):
    from concourse.bass_utils import run_bass_kernel_spmd

    x = np.asarray(x, dtype=np.float32)
    W_attn = np.asarray(W_attn, dtype=np.float32)
    b_attn = np.asarray(b_attn, dtype=np.float32)
    W_proj = np.asarray(W_proj, dtype=np.float32)
    b_proj = np.asarray(b_proj, dtype=np.float32)

    if "nc" not in _CACHE:
        _CACHE["nc"] = _build_nc()
    nc = _CACHE["nc"]

    in_maps = _shard_inputs(x, W_attn, b_attn, W_proj)
    res = run_bass_kernel_spmd(nc, in_maps, list(range(8)), trace=_trace)
    _CACHE["last_result"] = res

    out = np.empty((B, T, C), dtype=np.float32)
    for b in range(B):
        out[b] = res.results[2 * b]["out"] + res.results[2 * b + 1]["out"] + b_proj
    return out



# revision 3
# speedup vs baseline: 1.1274x; 1.1274x over previous
"""Causal self-attention (B=4, T=2048, C=768, H=12) on 8 TRN2 NeuronCores.

Sharding: (batch x head-half). Core c handles batch b = c//2 and heads
hg*6..hg*6+5 where hg = c%2. Each core computes the qkv projection for its
1152 W_attn columns, causal attention for its 6 heads, and a partial
c_proj using its 384 rows of W_proj. Host sums the pair partials + b_eff.

Key structure (v2):
- Heads processed in pairs: head-even on PE row-groups 0-1 (partitions
  0-63), head-odd on row-groups 2-3 (partitions 64-127), so the K=64
  score matmuls of a pair run concurrently in the PE array.
- Bias algebra: K-projection bias is dropped (softmax-invariant), the
  V bias is folded host-side into b_eff = b_proj + b_v @ W_proj, and the
  Q bias enters as exp(S/8 + (bq.K)/8) = exp(S/8)*exp(bqK/8) with
  exp(bqK/8) folded into the PV stationary operand (V rows and the
  sums column are scaled by it on the device).
- Tight causal diagonal: for diagonal 128-key blocks only the valid
  query range is computed/exp'd, packed contiguously in PSUM so one
  activation covers exactly the valid region.
- PV matmul carries the scaled-ones column FIRST, so row 0 of the PV
  accumulator is the softmax denominator: reciprocal on partition 0
  (DVE), partition-broadcast (Pool), fused evac-multiply (DVE).
- c_proj contracts head pairs at K=128 (half the matmuls).
- The triangle masks run on the Pool engine (affine_select), keeping
  DVE free for PSUM evacuations.
"""

import sys

import numpy as np

try:
    import concourse  # noqa: F401
except ImportError:
    sys.path.insert(0, "/opt/trn_rl_repo")

B, T, C, H, D = 4, 2048, 768, 12, 64
NH = H // 2          # 6 heads per core
CH = NH * D          # 384 channels per core
NCB = C // 128       # 6 contraction blocks
NTB = T // 128       # 16 t-blocks
NQC = T // 512       # 4 query chunks
NPAIR = NH // 2      # 3 head pairs
VW2 = D + 2          # 66: [V(64), eb, pad] per head (col 64 = eb = exp(bqK/8))
VROW = NH * VW2      # 396

_CACHE = {}


def _build_nc():
    from concourse import bacc, mybir, tile

    f32 = mybir.dt.float32
    bf16 = mybir.dt.bfloat16
    AF = mybir.ActivationFunctionType
    ALU = mybir.AluOpType

    nc = bacc.Bacc("TRN2", target_bir_lowering=False, debug=False, num_devices=8)

    xt_d = nc.dram_tensor("xt", [C, T], bf16, kind="ExternalInput")
    wqk_d = nc.dram_tensor("wqk", [C, 2 * CH], bf16, kind="ExternalInput")
    wv_d = nc.dram_tensor("wv", [C, CH + NH], bf16, kind="ExternalInput")
    wp_d = nc.dram_tensor("wp", [128, NPAIR * C], bf16, kind="ExternalInput")
    out_d = nc.dram_tensor("out", [T, C], f32, kind="ExternalOutput")

    with tile.TileContext(nc) as tc:
        with (
            tc.tile_pool(name="const", bufs=1) as cp,
            tc.tile_pool(name="wk", bufs=2) as wk,
            tc.tile_pool(name="pt", bufs=2) as ptp,
            tc.tile_pool(name="ot", bufs=2) as otp,
            tc.tile_pool(name="outs", bufs=2) as osp,
            tc.tile_pool(name="ps", bufs=3, space="PSUM") as psp,
            tc.tile_pool(name="pv", bufs=2, space="PSUM") as pvp,
        ):
            # ---- resident inputs ----
            xt_r = xt_d.rearrange("(n p) m -> n p m", p=128)
            wqk_r = wqk_d.rearrange("(n p) m -> n p m", p=128)
            wv_r = wv_d.rearrange("(n p) m -> n p m", p=128)
            xt_t, wqk_t, wv_t = [], [], []
            for ci in range(NCB):
                t_ = cp.tile([128, T], bf16, tag=f"xt{ci}")
                nc.sync.dma_start(out=t_, in_=xt_r[ci])
                xt_t.append(t_)
                t_ = cp.tile([128, 2 * CH], bf16, tag=f"wqk{ci}")
                nc.sync.dma_start(out=t_, in_=wqk_r[ci])
                wqk_t.append(t_)
                t_ = cp.tile([128, CH + NH], bf16, tag=f"wv{ci}")
                nc.sync.dma_start(out=t_, in_=wv_r[ci])
                wv_t.append(t_)
            wp_sb = cp.tile([128, NPAIR, C], bf16, tag="wp")
            nc.sync.dma_start(out=wp_sb, in_=wp_d.rearrange("p (n m) -> p n m", n=NPAIR))

            qkT = cp.tile([128, 6, T], bf16, tag="qkT")  # 0-2: Q pairs, 3-5: K pairs
            v1 = cp.tile([128, NTB, VROW], bf16, tag="v1")
            v1_4d = v1.rearrange("p n (h e) -> p n h e", e=VW2)

            for j in range(NQC):
                # ---------- projection work needed by this chunk ----------
                def qk_proj(p_):
                    for co in (p_, 3 + p_):
                        ps = psp.tile([128, 1024], f32, tag="s")
                        for ci in range(NCB):
                            nc.tensor.matmul(
                                ps[:, 0:512],
                                lhsT=wqk_t[ci][:, co * 128:(co + 1) * 128],
                                rhs=xt_t[ci][:, j * 512:(j + 1) * 512],
                                start=(ci == 0),
                                stop=(ci == NCB - 1),
                            )
                        nc.vector.tensor_copy(
                            qkT[:, co, j * 512:(j + 1) * 512], ps[:, 0:512]
                        )

                qk_proj(0)
                # V (+ bqK) for the 4 key blocks this chunk adds
                for tb in range(4 * j, 4 * j + 4):
                    psv = psp.tile([128, 1024], f32, tag="s")
                    for ci in range(NCB):
                        nc.tensor.matmul(
                            psv[:, 0:CH + NH],
                            lhsT=xt_t[ci][:, tb * 128:(tb + 1) * 128],
                            rhs=wv_t[ci],
                            start=(ci == 0),
                            stop=(ci == NCB - 1),
                        )
                    eb = wk.tile([128, NH], f32, tag="eb")
                    nc.scalar.activation(eb, psv[:, CH:CH + NH], AF.Exp, scale=0.125)
                    eb3 = eb.rearrange("p (h o) -> p h o", o=1)
                    nc.vector.tensor_mul(
                        v1_4d[:, tb, :, 0:D],
                        psv[:, 0:CH].rearrange("p (h e) -> p h e", e=D),
                        eb3.to_broadcast([128, NH, D]),
                    )
                    nc.vector.tensor_copy(v1_4d[:, tb, :, D:D + 1], eb3)

                # ---------- attention per head pair ----------
                ot_tiles = []
                nkb = 4 * (j + 1)
                for p in range(NPAIR):
                    if p > 0:
                        qk_proj(p)
                    ps_pv = [
                        pvp.tile([128, 512], f32, tag="pv", name=f"pspv{h2}")
                        for h2 in range(2)
                    ]
                    prev = None
                    for pti in range(nkb // 2):
                        kb0, kb1 = 2 * pti, 2 * pti + 1
                        d0 = kb0 * 128 - j * 512
                        d1 = d0 + 128
                        qlo0, qlo1 = max(d0, 0), max(d1, 0)
                        w0, w1 = 512 - qlo0, 512 - qlo1
                        cur = []
                        for h2 in range(2):
                            hp = h2 * 64
                            ps = psp.tile([128, 1024], f32, tag="s")
                            nc.tensor.matmul(
                                ps[:, 0:w0],
                                lhsT=qkT[hp:hp + 64, 3 + p, kb0 * 128:(kb0 + 1) * 128],
                                rhs=qkT[hp:hp + 64, p, j * 512 + qlo0:(j + 1) * 512],
                                start=True,
                                stop=True,
                            )
                            nc.tensor.matmul(
                                ps[:, w0:w0 + w1],
                                lhsT=qkT[hp:hp + 64, 3 + p, kb1 * 128:(kb1 + 1) * 128],
                                rhs=qkT[hp:hp + 64, p, j * 512 + qlo1:(j + 1) * 512],
                                start=True,
                                stop=True,
                            )
                            pt = ptp.tile([128, 1024], bf16, tag=f"pt{h2}")
                            nc.scalar.activation(
                                pt[:, 0:w0 + w1], ps[:, 0:w0 + w1], AF.Exp, scale=0.125
                            )
                            if d0 >= 0:
                                nc.gpsimd.affine_select(
                                    out=pt[:, 0:128],
                                    in_=pt[:, 0:128],
                                    compare_op=ALU.is_ge,
                                    fill=0.0,
                                    base=0,
                                    pattern=[[1, 128]],
                                    channel_multiplier=-1,
                                )
                            if d1 >= 0:
                                nc.gpsimd.affine_select(
                                    out=pt[:, w0:w0 + 128],
                                    in_=pt[:, w0:w0 + 128],
                                    compare_op=ALU.is_ge,
                                    fill=0.0,
                                    base=0,
                                    pattern=[[1, 128]],
                                    channel_multiplier=-1,
                                )
                            cur.append(pt)
                        if prev is not None:
                            _emit_pv(nc, prev, ps_pv, v1_4d, p, pti - 1, j, nkb)
                        prev = cur
                    _emit_pv(nc, prev, ps_pv, v1_4d, p, nkb // 2 - 1, j, nkb)

                    # normalization -> paired ot tile
                    otpair = otp.tile([128, 512], bf16, tag=f"ot{p}")
                    for h2 in range(2):
                        # sums live at partition 64; custom-DVE ops misread
                        # base-64 APs, so standard-copy to partition 0 first
                        sums_sb = wk.tile([1, 512], f32, tag="sums")
                        nc.vector.tensor_copy(sums_sb, ps_pv[h2][D:D + 1, :])
                        rb1 = wk.tile([1, 512], f32, tag="rb1")
                        nc.vector.reciprocal_approx_fast(rb1, sums_sb)
                        rbb = wk.tile([64, 512], f32, tag="rbb")
                        nc.gpsimd.partition_broadcast(rbb, rb1)
                        nc.vector.tensor_mul(
                            otpair[h2 * 64:(h2 + 1) * 64, :],
                            ps_pv[h2][0:D, :],
                            rbb,
                        )
                    ot_tiles.append(otpair)

                # ---------- partial c_proj for this chunk ----------
                for tb4 in range(4):
                    tb = j * 4 + tb4
                    pp = psp.tile([128, 1024], f32, tag="s")
                    for lo, n in ((0, 512), (512, 256)):
                        for p in range(NPAIR):
                            nc.tensor.matmul(
                                pp[:, lo:lo + n],
                                lhsT=ot_tiles[p][:, tb4 * 128:(tb4 + 1) * 128],
                                rhs=wp_sb[:, p, lo:lo + n],
                                start=(p == 0),
                                stop=(p == NPAIR - 1),
                            )
                    ost = osp.tile([128, C], f32, tag="ost")
                    nc.vector.tensor_copy(ost, pp[:, 0:C])
                    nc.sync.dma_start(out=out_d[tb * 128:(tb + 1) * 128, :], in_=ost)

    nc.compile()
    return nc


def _emit_pv(nc, pts, ps_pv, v1_4d, p, pti, j, nkb):
    """PV accumulation for kb pair (2*pti, 2*pti+1) of both heads."""
    for h2 in range(2):
        pt = pts[h2]
        h = 2 * p + h2  # head within core (0..5) -> v1 group
        for half, kb in ((0, 2 * pti), (1, 2 * pti + 1)):
            d = kb * 128 - j * 512
            qlo = max(d, 0)
            w = 512 - qlo
            x0 = 0 if half == 0 else (512 - max(2 * pti * 128 - j * 512, 0))
            nc.tensor.matmul(
                ps_pv[h2][0:1 + D, qlo:512],
                lhsT=v1_4d[:, kb, h, 0:1 + D],
                rhs=pt[:, x0:x0 + w],
                start=(kb == 0),
                stop=(kb == nkb - 1),
            )


def _bf16(a):
    import ml_dtypes
    return np.ascontiguousarray(a).astype(ml_dtypes.bfloat16)


def _shard_inputs(x, W_attn, b_attn, W_proj):
    in_maps = []
    for c in range(8):
        b, hg = c // 2, c % 2
        q0, k0, v0 = hg * CH, C + hg * CH, 2 * C + hg * CH
        # per-head bqK column: (Wk_h @ bq_h) -> scores bias via exp-fold
        bcols = np.stack(
            [
                W_attn[:, k0 + h * D:k0 + (h + 1) * D]
                @ b_attn[q0 + h * D:q0 + (h + 1) * D]
                for h in range(NH)
            ],
            axis=1,
        )  # [C, 6]
        in_maps.append({
            "xt": _bf16(x[b].T),
            "wqk": _bf16(np.concatenate(
                [W_attn[:, q0:q0 + CH], W_attn[:, k0:k0 + CH]], axis=1)),
            "wv": _bf16(np.concatenate(
                [W_attn[:, v0:v0 + CH], bcols], axis=1)),
            "wp": _bf16(
                W_proj[hg * CH:(hg + 1) * CH, :]
                .reshape(NPAIR, 128, C)
                .transpose(1, 0, 2)
                .reshape(128, NPAIR * C)
            ),
        })
    return in_maps


def kernel(x, W_attn, b_attn, W_proj, b_proj, _trace=False):
    from concourse.bass_utils import run_bass_kernel_spmd

    x = np.asarray(x, dtype=np.float32)
    W_attn = np.asarray(W_attn, dtype=np.float32)
    b_attn = np.asarray(b_attn, dtype=np.float32)
    W_proj = np.asarray(W_proj, dtype=np.float32)
    b_proj = np.asarray(b_proj, dtype=np.float32)

    if "nc" not in _CACHE:
        _CACHE["nc"] = _build_nc()
    nc = _CACHE["nc"]

    in_maps = _shard_inputs(x, W_attn, b_attn, W_proj)
    res = run_bass_kernel_spmd(nc, in_maps, list(range(8)), trace=_trace)
    _CACHE["last_result"] = res

    # V-bias contribution is a constant row: b_eff = b_proj + b_v @ W_proj
    b_eff = b_proj + b_attn[2 * C:] @ W_proj
    out = np.empty((B, T, C), dtype=np.float32)
    for b in range(B):
        out[b] = res.results[2 * b]["out"] + res.results[2 * b + 1]["out"] + b_eff
    return out
